# revision 50
# baseline (speedup 1.0000x reference)
"""Trainium2 Bass kernel for the octonion causal self-attention block.

Strategy (8 NeuronCores, SPMD, tensor-parallel over octonion components):
  Each core owns one octonion component c (= heads 2c, 2c+1).

  Host prep: ternary-quantize weights exactly as the reference does and keep
  them UNIT-scale ({-1,0,1}) so q/k weights are exactly representable in
  fp8-e4m3; the data-dependent ternary scales travel in a tiny [128,4] input
  tensor and are folded into the exp scale (sq*sk/sqrt(HD)) and the final
  output-copy scale (sv*so) -- no recompile if weights change.

  Device:
  - q/k projections run in fp8 DoubleRow (two 128-k-planes per matmul, 2x PE
    throughput); v projection stays bf16 (accuracy).  RoPE applied on the fly
    (partition-half swap via DMA).
  - Attention: S^T = K^T Q per s-tile; exp is computed mostly on DVE via a
    Schraudolph bitcast (round(S*128/ln2 + 16255.5) as int16 == bf16 bits of
    exp(S), +-6% on P which softmax normalization tolerates), with every 3rd
    chunk on ACT as real exp to balance engines.  P^T V with an appended
    ones-column gives the softmax denominator for free; per-partition
    normalize, PE-transpose back to channel-major yT.
  - Partial output projection per core (bf16); host sums the 8 partials.
  - Emission is pair-pipelined: phase-A (scores+exp) of attention pair p is
    interleaved with phase-B (PV+normalize+transpose) of pair p-1 and with
    leftover projection chunks / o-proj units, so the in-order PE queue always
    has ready work behind any stalled instruction.
"""

import numpy as np
import ml_dtypes

import concourse.bass as bass
import concourse.tile as tile
from concourse import bacc, mybir
from concourse.bass_utils import run_bass_kernel_spmd
from concourse.masks import make_identity

# ---------------------------------------------------------------- problem dims
B, T_FULL, C, H = 2, 2048, 2048, 16
HD = C // H          # 128
P = C // 8           # 256
N_CORES = 8
KT = C // 128        # 16 contraction k-tiles

SCHR_K = float(128.0 / np.log(2.0))   # schraudolph multiplier for bf16 bitcast
SCHR_B = 16255.5                      # 127<<7 - 0.5 rounding bias

OCT_SIGN = np.array([
    [1, 1, 1, 1, 1, 1, 1, 1],
    [1,-1, 1,-1, 1,-1,-1, 1],
    [1,-1,-1, 1, 1, 1,-1,-1],
    [1, 1,-1,-1, 1,-1, 1,-1],
    [1,-1,-1,-1,-1, 1, 1, 1],
    [1, 1,-1, 1,-1,-1,-1, 1],
    [1, 1, 1,-1,-1, 1,-1,-1],
    [1,-1, 1, 1,-1,-1, 1,-1]], dtype=np.float32)
OCT_IDX = np.array([
    [0,1,2,3,4,5,6,7],
    [1,0,3,2,5,4,7,6],
    [2,3,0,1,6,7,4,5],
    [3,2,1,0,7,6,5,4],
    [4,5,6,7,0,1,2,3],
    [5,4,7,6,1,0,3,2],
    [6,7,4,5,2,3,0,1],
    [7,6,5,4,3,2,1,0]], dtype=np.int32)
_COMB = np.zeros((8, 8, 8), np.float32)
for _i in range(8):
    for _j in range(8):
        _COMB[OCT_IDX[_i, _j], _i, _j] = OCT_SIGN[_i, _j]

BF16 = ml_dtypes.bfloat16
E4M3 = ml_dtypes.float8_e4m3   # TRN FP8_EXP4 (max 240), == mybir float8e4


# ------------------------------------------------------------------- host prep
def _ternary_scale(W: np.ndarray) -> np.float32:
    """The reference's ternary scale s = mean|W| + 1e-8 (f32 semantics)."""
    try:
        import jax
        import jax.numpy as jnp
        cpu = jax.local_devices(backend="cpu")[0]
        with jax.default_device(cpu):
            s = jnp.mean(jnp.abs(jnp.asarray(W))) + 1e-8
            return np.float32(np.asarray(s))
    except Exception:
        return np.float32(np.mean(np.abs(W.astype(np.float32)))) + np.float32(1e-8)


def _build_w_unit(W: np.ndarray) -> tuple[np.ndarray, np.float32]:
    """[8,P,P] weights -> (unit-scale effective [C,C] in {-1,0,1}, scale)."""
    s = _ternary_scale(W)
    q = np.rint(np.clip(W / s, -1.0, 1.0)).astype(np.float32)  # (8,P,P) +-1/0
    w_eff = np.einsum("kij,ipq->jpkq", _COMB, q).reshape(C, C)
    return w_eff, s


def _rope_colperm() -> np.ndarray:
    """colperm[new] = old: within each head, [re0..re63 | im0..im63]."""
    perm = np.zeros(C, dtype=np.int64)
    for h in range(H):
        base = h * HD
        for r in range(HD // 2):
            perm[base + r] = base + 2 * r
            perm[base + HD // 2 + r] = base + 2 * r + 1
    return perm


def prep_inputs(inputs: dict, T: int) -> list[dict]:
    """Build the 8 per-core input maps from the full problem inputs."""
    NT = B * T
    x = np.asarray(inputs["x"], np.float32)[:, :T, :]
    cos = np.asarray(inputs["freqs_cos"], np.float32)[:T]   # [T, 64]
    sin = np.asarray(inputs["freqs_sin"], np.float32)[:T]

    wq_u, sq = _build_w_unit(np.asarray(inputs["wq"], np.float32))
    wk_u, sk = _build_w_unit(np.asarray(inputs["wk"], np.float32))
    wv_u, sv = _build_w_unit(np.asarray(inputs["wv"], np.float32))
    wo_u, so = _build_w_unit(np.asarray(inputs["wo"], np.float32))

    perm = _rope_colperm()
    wq_u = wq_u[:, perm]
    wk_u = wk_u[:, perm]

    # data-dependent scalars, replicated per partition: [128, 4] f32
    exp_scale = np.float32(sq) * np.float32(sk) * np.float32(HD) ** np.float32(-0.5)
    scal = np.zeros((128, 4), np.float32)
    scal[:, 2] = np.float32(sv) * np.float32(so) # o-proj output scale
    # q is pre-scaled by K*sq*sk/sqrt(HD) during its PSUM->SBUF copy, so the
    # scores PSUM is already in schraudolph domain (S_true * 128/ln2)
    scal[:, 3] = np.float32(SCHR_K) * exp_scale

    xf = x.reshape(NT, C)
    # xT k-tiles: bf16 (v-proj) and fp8 (q/k-proj)
    xt = np.ascontiguousarray(xf.T.reshape(KT, 128, NT).astype(BF16))
    xtq = np.ascontiguousarray(xf.T.reshape(KT, 128, NT).astype(E4M3))

    # plain rope tables, duplicated-half layout [128, T]
    cosd = np.empty((128, T), np.float32)
    cosd[0:64] = cos.T
    cosd[64:128] = cos.T
    sind = np.empty((128, T), np.float32)
    sind[0:64] = -sin.T
    sind[64:128] = sin.T
    cosd = cosd.astype(BF16)
    sind = sind.astype(BF16)

    tri = np.triu(np.ones((128, 128), np.float32)).astype(BF16)  # [s,q] s<=q

    def blocks(w_eff: np.ndarray, c: int, dt) -> np.ndarray:
        blk = w_eff[:, c * P:(c + 1) * P]                  # [C, 256]
        return np.ascontiguousarray(blk.reshape(KT, 128, P).astype(dt))

    in_maps = []
    for c in range(N_CORES):
        wo_rows = np.ascontiguousarray(
            wo_u[c * P:(c + 1) * P, :].reshape(2, 128, C).astype(BF16))
        in_maps.append({
            "xt": xt,
            "xtq": xtq,
            "wq": blocks(wq_u, c, E4M3),
            "wk": blocks(wk_u, c, E4M3),
            "wv": blocks(wv_u, c, BF16),
            "wo": wo_rows,
            "cosd": cosd,
            "sind": sind,
            "tri": tri,
            "scal": scal,
        })
    return in_maps


# ------------------------------------------------------------- device program
def build_nc(T: int = T_FULL, n_cores: int = N_CORES):
    NT = B * T
    ST = T // 128            # s-tiles per batch
    NST = NT // 128
    TCH = min(512, T)        # token chunk; must not cross a batch boundary
    VCH = TCH // 2           # v-proj half-chunk (smaller bf16 x tiles)
    NCH = NT // TCH
    CPB = T // TCH           # chunks per batch
    bf16 = mybir.dt.bfloat16
    f32 = mybir.dt.float32
    fp8 = mybir.dt.float8e4
    i16 = mybir.dt.int16

    nc = bacc.Bacc("TRN2", target_bir_lowering=False, debug=False,
                   num_devices=n_cores)

    xt_d = nc.dram_tensor("xt", [KT, 128, NT], bf16, kind="ExternalInput")
    xtq_d = nc.dram_tensor("xtq", [KT, 128, NT], fp8, kind="ExternalInput")
    wq_d = nc.dram_tensor("wq", [KT, 128, P], fp8, kind="ExternalInput")
    wk_d = nc.dram_tensor("wk", [KT, 128, P], fp8, kind="ExternalInput")
    wv_d = nc.dram_tensor("wv", [KT, 128, P], bf16, kind="ExternalInput")
    wo_d = nc.dram_tensor("wo", [2, 128, C], bf16, kind="ExternalInput")
    cos_d = nc.dram_tensor("cosd", [128, T], bf16, kind="ExternalInput")
    sin_d = nc.dram_tensor("sind", [128, T], bf16, kind="ExternalInput")
    tri_d = nc.dram_tensor("tri", [128, 128], bf16, kind="ExternalInput")
    scal_d = nc.dram_tensor("scal", [128, 4], f32, kind="ExternalInput")
    out_d = nc.dram_tensor("outt", [C, NT], bf16, kind="ExternalOutput")

    with tile.TileContext(nc) as tc:
        with (
            tc.tile_pool(name="consts", bufs=1) as consts,
            tc.tile_pool(name="persist", bufs=1) as persist,
            tc.tile_pool(name="pt", bufs=1) as pt_pool,
            tc.tile_pool(name="sc", bufs=3, space="PSUM") as ps_sc,
            tc.tile_pool(name="osc", bufs=2, space="PSUM") as ps_oc,
        ):
            # ---- resident constants (DMA order == consumption order)
            # wo/ident live in a late pool opened after the x-chunk pools
            # close, to keep the phase-1 SBUF peak under the limit.
            wq_s = consts.tile([128, KT, P], fp8, tag="wq")
            wk_s = consts.tile([128, KT, P], fp8, tag="wk")
            wv_s = consts.tile([128, KT, P], bf16, tag="wv")
            cos_s = consts.tile([128, T], bf16, tag="cos")
            sin_s = consts.tile([128, T], bf16, tag="sin")
            tri_s = consts.tile([128, 128], bf16, tag="tri")
            scal_s = consts.tile([128, 4], f32, tag="scal")

            # persistent activations
            qt_s = persist.tile([128, 2, NT], bf16, tag="qt")  # [d, head, tok]
            kt_s = persist.tile([128, 2, NT], bf16, tag="kt")
            v_s = persist.tile([128, NST, 2, 130], bf16, tag="v")
            ystages = {
                (b, a): persist.tile([128, T], bf16, tag=f"ystage{b}{a}",
                                     name=f"ystage{b}{a}")
                for b in range(B) for a in range(2)
            }

            exp_ct = [0]   # round-robin counter for ACT/DVE exp split

            # ============ emission helpers (units are small closures) =======
            def emit_preamble(xtq_tiles, xt_tiles):
                # critical path: the first DoubleRow matmul needs the first
                # wq k-quarter + first xtq k-quarter.  Split the trigger
                # streams across the three DGE-capable queues so descriptor
                # generation runs in parallel:
                #   sync:   wq quarters, wk, rope tables
                #   gpsimd: xtq-chunk0 quarters, xtq-chunk1 prefetch, tri
                #   scalar: scal, xt halves, wv (v-proj inputs)
                nc.scalar.dma_start(out=scal_s, in_=scal_d.ap())
                for kq in range(0, KT, 4):
                    nc.sync.dma_start(
                        out=wq_s[:, kq:kq + 4, :],
                        in_=wq_d.ap()[kq:kq + 4].rearrange("k p n -> p k n"))
                    nc.gpsimd.dma_start(
                        out=xtq_tiles[0][:, kq:kq + 4, :],
                        in_=xtq_d.ap()[kq:kq + 4, :, 0:TCH]
                        .rearrange("k p n -> p k n"))
                nc.sync.dma_start(out=wk_s,
                                  in_=wk_d.ap().rearrange("k p n -> p k n"))
                nc.sync.dma_start(out=cos_s, in_=cos_d.ap())
                nc.sync.dma_start(out=sin_s, in_=sin_d.ap())
                # chunk-0 bf16 x halves + wv (v-proj) on the scalar queue
                nc.scalar.dma_start(
                    out=xt_tiles[0],
                    in_=xt_d.ap()[:, :, 0:VCH].rearrange("k p n -> p k n"))
                nc.scalar.dma_start(out=wv_s,
                                    in_=wv_d.ap().rearrange("k p n -> p k n"))
                nc.scalar.dma_start(
                    out=xt_tiles[1],
                    in_=xt_d.ap()[:, :, VCH:2 * VCH]
                    .rearrange("k p n -> p k n"))
                nc.gpsimd.dma_start(out=xtq_tiles[1],
                                    in_=xtq_d.ap()[:, :, TCH:2 * TCH]
                                    .rearrange("k p n -> p k n"))
                nc.gpsimd.dma_start(out=xtq_tiles[2],
                                    in_=xtq_d.ap()[:, :, 2 * TCH:3 * TCH]
                                    .rearrange("k p n -> p k n"))
                nc.gpsimd.dma_start(out=tri_s, in_=tri_d.ap())
                nc.vector.memset(v_s[:, :, :, 128:129], 1.0)

            def emit_chunk(ch, xtq_pool, xts_pool, rope_pool, ps_v,
                           xtq_pre=None, xt_pre=None):
                """Projections + rope for one 512-token chunk."""
                t0 = ch * TCH
                pos0 = t0 % T
                if xtq_pre is not None:
                    xtq_s = xtq_pre
                else:
                    xtq_s = xtq_pool.tile([128, KT, TCH], fp8, tag="xtq")
                    nc.sync.dma_start(out=xtq_s,
                                      in_=xtq_d.ap()[:, :, t0:t0 + TCH]
                                      .rearrange("k p n -> p k n"))

                # q/k projections in fp8 DoubleRow + rope
                for w_s, dst in ((wq_s, qt_s), (wk_s, kt_s)):
                    for a in range(2):
                        ps_q = ps_oc.tile([128, TCH], f32, tag="osc")
                        for kk in range(KT // 2):
                            nc.tensor.matmul(
                                ps_q[:],
                                lhsT=w_s[:, 2 * kk:2 * kk + 2,
                                         a * 128:(a + 1) * 128],
                                rhs=xtq_s[:, 2 * kk:2 * kk + 2, :],
                                start=(kk == 0), stop=(kk == KT // 2 - 1),
                                perf_mode=mybir.MatmulPerfMode.DoubleRow)
                        q_sb = rope_pool.tile([128, TCH], bf16, tag="qsb")
                        if w_s is wq_s:
                            # fold K*sq*sk/sqrt(HD) into q here so the scores
                            # PSUM lands directly in schraudolph domain
                            nc.scalar.activation(
                                out=q_sb[:], in_=ps_q[:],
                                func=mybir.ActivationFunctionType.Copy,
                                scale=scal_s[:, 3:4])
                        else:
                            nc.scalar.copy(out=q_sb[:], in_=ps_q[:])
                        qsw = rope_pool.tile([128, TCH], bf16, tag="qsw")
                        # partition-half swap via the idle GpSimd DGE queue
                        nc.gpsimd.dma_start(out=qsw[0:64, :],
                                            in_=q_sb[64:128, :])
                        nc.gpsimd.dma_start(out=qsw[64:128, :],
                                            in_=q_sb[0:64, :])
                        t1 = rope_pool.tile([128, TCH], bf16, tag="t1")
                        nc.vector.tensor_mul(
                            t1[:], q_sb[:], cos_s[:, pos0:pos0 + TCH])
                        t2 = rope_pool.tile([128, TCH], bf16, tag="t2")
                        nc.vector.tensor_mul(
                            t2[:], qsw[:], sin_s[:, pos0:pos0 + TCH])
                        nc.vector.tensor_add(
                            dst[:, a, t0:t0 + TCH], t1[:], t2[:])

                # v projection (natural layout, bf16), 256-token halves
                for vh in range(TCH // VCH):
                    vt0 = t0 + vh * VCH
                    if xt_pre is not None and ch == 0:
                        xt_s = xt_pre[vh]
                    else:
                        xt_s = xts_pool.tile([128, KT, VCH], bf16, tag="xt")
                        nc.sync.dma_start(out=xt_s,
                                          in_=xt_d.ap()[:, :, vt0:vt0 + VCH]
                                          .rearrange("k p n -> p k n"))
                    for st in range(VCH // 128):
                        stg = vt0 // 128 + st
                        ps_vt = ps_v.tile([128, P], f32, tag="psv")
                        for k in range(KT):
                            nc.tensor.matmul(
                                ps_vt[:],
                                lhsT=xt_s[:, k, st * 128:(st + 1) * 128],
                                rhs=wv_s[:, k, :],
                                start=(k == 0), stop=(k == KT - 1))
                        # sv*so folded into v here (ACT, scaled copy) -- the
                        # o-proj output copies then need no scaling, and DVE
                        # stays free for the exp stream
                        nc.scalar.activation(
                            out=v_s[:, stg, :, 0:128],
                            in_=ps_vt[:].rearrange("p (a d) -> p a d", a=2),
                            func=mybir.ActivationFunctionType.Copy,
                            scale=scal_s[:, 2:3])

            def a_units(p, pts):
                """Phase A of pair p: scores^T -> exp -> mask, one unit per j."""
                b, a = p
                qh = qt_s[:, a, b * T:(b + 1) * T]
                kh = kt_s[:, a, b * T:(b + 1) * T]
                for j in range(ST):
                    def unit(j=j):
                        w_j = T - 128 * j
                        pt_j = pt_pool.tile([128, w_j], bf16,
                                            tag=f"pt{j}_{(p[0] * 2 + p[1]) % 2}",
                                            name=f"ptj{j}")
                        pts[j] = pt_j
                        q0 = 128 * j
                        while q0 < T:
                            w = min(512, T - q0)
                            ps = ps_sc.tile([128, TCH], f32, tag="sc")
                            nc.tensor.matmul(
                                ps[:, 0:w],
                                lhsT=kh[:, 128 * j:128 * (j + 1)],
                                rhs=qh[:, q0:q0 + w],
                                start=True, stop=True)
                            rel = q0 - 128 * j
                            if exp_ct[0] % 5 < 3:
                                # PSUM already holds S*128/ln2; exp(x/K) on ACT
                                nc.scalar.activation(
                                    out=pt_j[:, rel:rel + w], in_=ps[:, 0:w],
                                    func=mybir.ActivationFunctionType.Exp,
                                    scale=float(1.0 / SCHR_K))
                            else:
                                # single-ALU-pass schraudolph: add bias, cast
                                # to int16, reinterpret as bf16 == exp(S)
                                nc.vector.tensor_scalar_add(
                                    pt_j[:, rel:rel + w].bitcast(i16),
                                    ps[:, 0:w], SCHR_B)
                            exp_ct[0] += 1
                            q0 += w
                        # causal mask on the diagonal 128x128 block (GpSimd --
                        # SBUF-only op, keeps DVE free for the exp stream)
                        nc.gpsimd.tensor_mul(
                            pt_j[:, 0:128], pt_j[:, 0:128], tri_s[:])
                    yield unit

            def b_units(p, pts, y_sbs):
                """Phase B chains of pair p (descending i): PV + normalize."""
                b, a = p
                for i in reversed(range(ST)):
                    def unit(i=i):
                        psy = ps_py.tile([128, 132], f32, tag="psy")
                        for j in range(i + 1):
                            nc.tensor.matmul(
                                psy[:, 0:129],
                                lhsT=pts[j][:, 128 * (i - j):128 * (i - j) + 128],
                                rhs=v_s[:, b * ST + j, a, 0:129],
                                start=(j == 0), stop=(j == i))
                        rc = small_pool.tile([128, 1], f32, tag="rc")
                        nc.vector.reciprocal(rc[:], psy[:, 128:129])
                        y_sb = ysb_pool.tile([128, 128], bf16, tag="ysb")
                        # normalize alternates DVE/ACT to balance the two
                        # PSUM-reading engines during attention
                        if i % 2 == 0:
                            nc.vector.tensor_scalar_mul(
                                y_sb[:], psy[:, 0:128], rc[:])
                        else:
                            nc.scalar.activation(
                                out=y_sb[:], in_=psy[:, 0:128],
                                func=mybir.ActivationFunctionType.Copy,
                                scale=rc[:])
                        y_sbs[i] = y_sb
                    yield unit

            def b_tail(p, y_sbs):
                """Transposes of pair p back to channel-major yT."""
                y_stage = ystages[p]
                for i in range(ST):
                    def unit(i=i):
                        pst = ps_tr.tile([128, 128], bf16, tag="pst")
                        nc.tensor.transpose(pst[:], y_sbs[i][:], ident[:])
                        # NB: must stay on DVE -- ACT reading bf16 PSUM
                        # hard-faulted the exec unit on HW
                        nc.vector.tensor_copy(
                            y_stage[:, 128 * i:128 * (i + 1)], pst[:])
                    yield unit

            def o_units(b):
                """Partial o-proj for batch b: units of 4 m-tiles staged into
                one SBUF block so each DMA trigger covers 4x the data."""
                for lch in range(CPB):
                    lt0 = lch * TCH
                    t0 = b * T + lt0
                    for mb in range(C // 512):
                        def unit(lt0=lt0, t0=t0, mb=mb):
                            o_sb = o_pool.tile([128, 4, TCH], bf16, tag="osb")
                            for mi in range(4):
                                m = mb * 4 + mi
                                ps = ps_oc.tile([128, TCH], f32, tag="osc")
                                for k in range(2):
                                    nc.tensor.matmul(
                                        ps[:, 0:TCH],
                                        lhsT=wo_s[:, k, m * 128:(m + 1) * 128],
                                        rhs=ystages[(b, k)][:, lt0:lt0 + TCH],
                                        start=(k == 0), stop=(k == 1))
                                # plain copies (sv*so already folded into v),
                                # alternating DVE/ACT
                                if mi % 2 == 0:
                                    nc.vector.tensor_copy(
                                        o_sb[:, mi, :], ps[:, 0:TCH])
                                else:
                                    nc.scalar.copy(
                                        out=o_sb[:, mi, :], in_=ps[:, 0:TCH])
                            # split the staged block across DMA queues to cut
                            # the serialized per-queue transfer time; the
                            # kernel's final chunk gets the finest splits
                            # since its transfers are pure tail latency
                            last_ch = (b == 1 and lt0 == 0)
                            if last_ch and mb == 3:
                                # 8 x 64KB: one m-tile x partition-half each
                                for mi in range(4):
                                    r0 = mb * 512 + mi * 128
                                    for ph in range(2):
                                        nc.sync.dma_start(
                                            out=out_d.ap()[
                                                r0 + 64 * ph:r0 + 64 * (ph + 1),
                                                t0:t0 + TCH],
                                            in_=o_sb[64 * ph:64 * (ph + 1),
                                                     mi, :])
                                return
                            nsp = 4 if last_ch else 2
                            w_m = 4 // nsp
                            for h in range(nsp):
                                r0 = mb * 512 + h * 128 * w_m
                                nc.sync.dma_start(
                                    out=out_d.ap()[r0:r0 + 128 * w_m,
                                                   t0:t0 + TCH]
                                    .rearrange("(m p) n -> p m n", m=w_m),
                                    in_=o_sb[:, w_m * h:w_m * (h + 1), :])
                        yield unit

            def zip_emit(*streams):
                """Round-robin emit units from several unit-iterators."""
                streams = [iter(s) for s in streams]
                while streams:
                    nxt = []
                    for s in streams:
                        u = next(s, None)
                        if u is not None:
                            u()
                            nxt.append(s)
                    streams = nxt

            # ======================= emission schedule ======================
            PAIRS = [(0, 0), (0, 1), (1, 0), (1, 1)]
            pts = {p: [None] * ST for p in PAIRS}
            ysbs = {p: [None] * ST for p in PAIRS}

            with (
                nc.named_scope("proj"),
                tc.tile_pool(name="xtq", bufs=3) as xtq_pool,
                tc.tile_pool(name="xts", bufs=2) as xts_pool,
                tc.tile_pool(name="rope", bufs=2) as rope_pool,
                tc.tile_pool(name="psv", bufs=2, space="PSUM") as ps_v,
            ):
                xtq0 = xtq_pool.tile([128, KT, TCH], fp8, tag="xtq")
                xtq1 = xtq_pool.tile([128, KT, TCH], fp8, tag="xtq")
                xtq2 = xtq_pool.tile([128, KT, TCH], fp8, tag="xtq")
                xt0a = xts_pool.tile([128, KT, VCH], bf16, tag="xt")
                xt0b = xts_pool.tile([128, KT, VCH], bf16, tag="xt")
                emit_preamble([xtq0, xtq1, xtq2], [xt0a, xt0b])
                pre_xtq = {0: xtq0, 1: xtq1, 2: xtq2}
                for ch in range(CPB):   # batch 0
                    emit_chunk(ch, xtq_pool, xts_pool, rope_pool, ps_v,
                               xtq_pre=pre_xtq.get(ch),
                               xt_pre=[xt0a, xt0b] if ch == 0 else None)
                # A(P0) zipped with the batch-1 projection chunks
                def chunk_units():
                    for ch in range(CPB, NCH):
                        def unit(ch=ch):
                            emit_chunk(ch, xtq_pool, xts_pool, rope_pool, ps_v)
                        yield unit
                zip_emit(a_units(PAIRS[0], pts[PAIRS[0]]), chunk_units())

            with (
                nc.named_scope("attn"),
                tc.tile_pool(name="late", bufs=1) as late,
                tc.tile_pool(name="small", bufs=4) as small_pool,
                tc.tile_pool(name="ysb", bufs=ST + 4) as ysb_pool,
                tc.tile_pool(name="ostage", bufs=4) as o_pool,
                tc.tile_pool(name="psy", bufs=2, space="PSUM") as ps_py,
                tc.tile_pool(name="pst", bufs=1, space="PSUM") as ps_tr,
            ):
                wo_s = late.tile([128, 2, C], bf16, tag="wo")
                nc.sync.dma_start(out=wo_s,
                                  in_=wo_d.ap().rearrange("k p n -> p k n"))
                ident = late.tile([128, 128], bf16, tag="ident")
                make_identity(nc, ident[:])
                P0, P1, P2, P3 = PAIRS

                def b_stream(p):
                    # descending chains with each transpose emitted two
                    # chains after its ts_mul: keeps PE fed without making
                    # a transpose head-of-line-block on a pending DVE op
                    chains = list(b_units(p, pts[p], ysbs[p]))   # i=15..0
                    tails = list(b_tail(p, ysbs[p]))             # i=0..15
                    seq = [chains[0]]
                    for idx in range(1, ST):
                        seq.append(chains[idx])
                        seq.append(tails[ST - idx])
                    seq.append(tails[0])
                    return seq

                # A(p+1) zipped with B(p) (+tails)
                zip_emit(a_units(P1, pts[P1]), b_stream(P0))
                zip_emit(a_units(P2, pts[P2]), b_stream(P1))
                # o-proj(b0) is ready after B(P1); spread it across the two
                # remaining attention steps
                ou0 = list(o_units(0))
                zip_emit(a_units(P3, pts[P3]), b_stream(P2), ou0[0:8])
                zip_emit(list(b_units(P3, pts[P3], ysbs[P3])), ou0[8:16])
                # interleave P3's transposes with o-proj(b1), descending
                # chunk order to match the descending chain emission (high-i
                # ts_muls complete first)
                tails3 = list(b_tail(P3, ysbs[P3]))
                ob1 = list(o_units(1))
                for lch in reversed(range(CPB)):
                    for u in tails3[4 * lch:4 * lch + 4]:
                        u()
                    for u in ob1[4 * lch:4 * lch + 4]:
                        u()

    nc.compile()
    return nc


# ------------------------------------------------------------------ entrypoint
_NC_CACHE: dict = {}


def _get_nc(T: int):
    if T not in _NC_CACHE:
        _NC_CACHE[T] = build_nc(T)
    return _NC_CACHE[T]


def assemble_output(results: list[dict], T: int = T_FULL) -> np.ndarray:
    # unshard = sum of the 8 tensor-parallel partial projections (bf16 -> f32)
    outT = results[0]["outt"].astype(np.float32)                # [C, NT]
    for r in results[1:]:
        outT += r["outt"].astype(np.float32)
    return np.ascontiguousarray(outT.T).reshape(B, T, C).astype(np.float32)


def kernel(**inputs) -> np.ndarray:
    nc = _get_nc(T_FULL)
    in_maps = prep_inputs(inputs, T_FULL)
    res = run_bass_kernel_spmd(nc, in_maps, list(range(N_CORES)))
    return assemble_output(res.results, T_FULL)
